# revision 1
# baseline (speedup 1.0000x reference)
"""Trainium2 Bass kernel for the Mamba-style CnvEncoder.

Sharding: data-parallel over batch — 8 batch rows, one per NeuronCore.
Each core runs the full pipeline for its row:
  in_proj -> in_proj2 (u,z) -> causal depthwise conv+SiLU -> x_proj ->
  dt_proj+softplus -> selective scan (tensor_tensor_scan per (d-block, n)) ->
  gate -> time-mean -> out_proj (folded to a single matvec since mean
  commutes with the linear out_proj).

kernel(**inputs) takes FULL unsharded inputs, returns the FULL (8, 768) output.
Self-contained: hardcodes all shapes; no file reads.
"""

import numpy as np
import ml_dtypes

import concourse.bacc as bacc
import concourse.mybir as mybir
from concourse.tile import TileContext
from concourse.bass_utils import run_bass_kernel_spmd
from concourse.masks import make_identity
from concourse.tile_rust import add_dep_helper

F32 = mybir.dt.float32
BF16 = mybir.dt.bfloat16
AL = mybir.AluOpType
AF = mybir.ActivationFunctionType

BF = ml_dtypes.bfloat16

# Model dims (hardcoded per problem spec)
B, L, DM, LAT = 8, 2048, 512, 768
DI, NST, DTR, DCONV = 1536, 16, 48, 4
Lc = 512                 # L-chunk (free-dim tile)
NCH = L // Lc            # 4
NB = DI // 128           # 12 d-blocks
KA = DM // 128           # 4
KH = LAT // 128          # 6
MH = LAT // 128          # 6


def build_nc(a_vals):
    """Build + compile the per-core program. a_vals: 16 floats (A[0,:])."""
    nc = bacc.Bacc("TRN2", target_bir_lowering=False, debug=False, num_devices=8)

    # DRAM I/O (per core)
    xT_d = nc.dram_tensor("xT", [DM, L], BF16, kind="ExternalInput")
    wip_d = nc.dram_tensor("wip", [DM, LAT], BF16, kind="ExternalInput")
    bip_d = nc.dram_tensor("bip", [128, MH], F32, kind="ExternalInput")
    win_d = nc.dram_tensor("win", [LAT, 2 * DI], BF16, kind="ExternalInput")
    cw_d = nc.dram_tensor("cw", [128, NB * DCONV], F32, kind="ExternalInput")
    cb_d = nc.dram_tensor("cb", [128, NB], F32, kind="ExternalInput")
    wxp_d = nc.dram_tensor("wxp", [DI, DTR + 2 * NST], BF16, kind="ExternalInput")
    wdt_d = nc.dram_tensor("wdt", [DTR, DI], BF16, kind="ExternalInput")
    bdt_d = nc.dram_tensor("bdt", [128, NB], F32, kind="ExternalInput")
    dv_d = nc.dram_tensor("dv", [128, NB], F32, kind="ExternalInput")
    sel_d = nc.dram_tensor("sel", [2 * NST, 2 * NST * 128], BF16,
                           kind="ExternalInput")
    out_d = nc.dram_tensor("out", [DI], F32, kind="ExternalOutput")

    with TileContext(nc) as tc:
        with (
            tc.tile_pool(name="const", bufs=1) as cp,
            tc.tile_pool(name="work", bufs=1) as wp,
            tc.tile_pool(name="ps", bufs=6, space="PSUM") as ps,
        ):
            # ---- constants / weights ----
            win_sb = []
            for k in range(KH):
                t = cp.tile([128, 2 * DI], BF16, tag=f"win{k}")
                nc.sync.dma_start(t[:], win_d.ap()[k * 128:(k + 1) * 128, :])
                win_sb.append(t)
            wip_sb = []
            for k in range(KA):
                t = cp.tile([128, LAT], BF16, tag=f"wip{k}")
                nc.sync.dma_start(t[:], wip_d.ap()[k * 128:(k + 1) * 128, :])
                wip_sb.append(t)
            wxp_sb = []
            for k in range(NB):
                t = cp.tile([128, DTR + 2 * NST], BF16, tag=f"wxp{k}")
                nc.sync.dma_start(t[:], wxp_d.ap()[k * 128:(k + 1) * 128, :])
                wxp_sb.append(t)
            wdt_sb = cp.tile([DTR, DI], BF16, tag="wdt")
            nc.sync.dma_start(wdt_sb[:], wdt_d.ap())
            bip_sb = cp.tile([128, MH], F32, tag="bip")
            nc.sync.dma_start(bip_sb[:], bip_d.ap())
            cw_sb = cp.tile([128, NB * DCONV], F32, tag="cw")
            nc.sync.dma_start(cw_sb[:], cw_d.ap())
            cb_sb = cp.tile([128, NB], F32, tag="cb")
            nc.sync.dma_start(cb_sb[:], cb_d.ap())
            bdt_sb = cp.tile([128, NB], F32, tag="bdt")
            nc.sync.dma_start(bdt_sb[:], bdt_d.ap())
            dv_sb = cp.tile([128, NB], F32, tag="dv")
            nc.sync.dma_start(dv_sb[:], dv_d.ap())

            idn = cp.tile([128, 128], BF16, tag="idn")
            make_identity(nc, idn[:])
            # row-selector blocks: sel[:, n*128:(n+1)*128] is one-hot row n
            sel = cp.tile([2 * NST, 2 * NST * 128], BF16, tag="sel")
            nc.sync.dma_start(sel[:], sel_d.ap())

            carry = cp.tile([128, NB * NST], F32, tag="carry")
            ycol = cp.tile([128, NCH * NB], F32, tag="ycol")
            halo = cp.tile([128, 3 * NB], F32, tag="halo")
            nc.gpsimd.memset(halo[:], 0.0)

            prev_last_exp = None  # ACT table-set ordering across chunks
            for c in range(NCH):
                csl = slice(c * Lc, (c + 1) * Lc)
                silu_chain = None
                # ---- A: in_proj  h = x @ w_ip.T + b_ip  -> [LAT, Lc] bf16
                xc = []
                for k in range(KA):
                    t = wp.tile([128, Lc], BF16, tag=f"xc{k}", bufs=2)
                    nc.sync.dma_start(t[:], xT_d.ap()[k * 128:(k + 1) * 128, csl])
                    xc.append(t)
                h_sb = []
                for m in range(MH):
                    ph = ps.tile([128, Lc], F32, tag="ps")
                    for k in range(KA):
                        nc.tensor.matmul(
                            ph[:], wip_sb[k][:, m * 128:(m + 1) * 128], xc[k][:],
                            start=(k == 0), stop=(k == KA - 1))
                    t = wp.tile([128, Lc], BF16, tag=f"h{m}", bufs=2)
                    nc.scalar.activation(t[:], ph[:], AF.Identity,
                                         bias=bip_sb[:, m:m + 1], scale=1.0)
                    h_sb.append(t)

                # ---- B: in_proj2  xz = h @ w_in.T ; u -> halo tiles, z -> silu
                uh_sb = []
                sz_sb = []
                for m in range(2 * NB):
                    pxz = ps.tile([128, Lc], F32, tag="ps")
                    for k in range(KH):
                        nc.tensor.matmul(
                            pxz[:], win_sb[k][:, m * 128:(m + 1) * 128], h_sb[k][:],
                            start=(k == 0), stop=(k == KH - 1))
                    if m < NB:
                        ut = wp.tile([128, Lc], F32, tag="u", bufs=6)
                        nc.scalar.activation(ut[:], pxz[:], AF.Identity,
                                             bias=0.0, scale=1.0)
                        uh_sb.append(ut)
                    else:
                        b = m - NB
                        t = wp.tile([128, Lc], BF16, tag=f"sz{b}")
                        i_s = nc.scalar.activation(t[:], pxz[:], AF.Silu)
                        # keep Silu-set ops before this chunk's Exp/Ln-set ops
                        # and after the previous chunk's (2 table loads/chunk)
                        if prev_last_exp is not None:
                            add_dep_helper(i_s.ins, prev_last_exp.ins,
                                           reason="ACT table grouping")
                        if silu_chain is not None:
                            add_dep_helper(i_s.ins, silu_chain.ins,
                                           reason="ACT table grouping")
                        silu_chain = i_s
                        sz_sb.append(t)

                # ---- C: causal depthwise conv + SiLU -> uc (bf16)
                # tap k reads u[t-3+k]; first (3-k) cols come from the halo
                uc_sb = []
                for b in range(NB):
                    ut = uh_sb[b]
                    hk = halo[:, 3 * b:3 * (b + 1)]
                    acc = wp.tile([128, Lc], F32, tag="cacc", bufs=4)
                    nc.vector.tensor_scalar_mul(
                        acc[:], ut[:], cw_sb[:, 4 * b + 3:4 * b + 4])
                    for k in range(DCONV - 1):
                        s = 3 - k
                        cwc = cw_sb[:, 4 * b + k:4 * b + k + 1]
                        acc2 = wp.tile([128, Lc], F32, tag="cacc", bufs=4)
                        nc.vector.scalar_tensor_tensor(
                            acc2[:, s:Lc], ut[:, 0:Lc - s], cwc,
                            acc[:, s:Lc], op0=AL.mult, op1=AL.add)
                        nc.vector.scalar_tensor_tensor(
                            acc2[:, 0:s], hk[:, k:k + s], cwc,
                            acc[:, 0:s], op0=AL.mult, op1=AL.add)
                        acc = acc2
                    # stash this chunk's last 3 u cols for the next chunk
                    nc.vector.tensor_copy(halo[:, 3 * b:3 * (b + 1)],
                                          ut[:, Lc - 3:Lc])
                    t = wp.tile([128, Lc], BF16, tag=f"uc{b}")
                    i_s = nc.scalar.activation(t[:], acc[:], AF.Silu,
                                               bias=cb_sb[:, b:b + 1], scale=1.0)
                    if prev_last_exp is not None:
                        add_dep_helper(i_s.ins, prev_last_exp.ins,
                                       reason="ACT table grouping")
                    if silu_chain is not None:
                        add_dep_helper(i_s.ins, silu_chain.ins,
                                       reason="ACT table grouping")
                    silu_chain = i_s
                    uc_sb.append(t)

                # ---- D: x_proj  dbc = uc @ w_xp.T  [80, Lc]
                pdbc = ps.tile([DTR + 2 * NST, Lc], F32, tag="ps")
                for k in range(NB):
                    nc.tensor.matmul(pdbc[:], wxp_sb[k][:], uc_sb[k][:],
                                     start=(k == 0), stop=(k == NB - 1))
                # host packs w_xp rows as [B(16); C(16); dt_lo(48)] so both
                # psum reads start at 32-aligned partitions
                dtlo = wp.tile([DTR, Lc], BF16, tag="dtlo", bufs=2)
                nc.scalar.activation(dtlo[0:32, :], pdbc[32:64, :],
                                     AF.Identity, bias=0.0, scale=1.0)
                nc.scalar.activation(dtlo[32:DTR, :], pdbc[64:32 + DTR, :],
                                     AF.Identity, bias=0.0, scale=1.0)
                bc_f = wp.tile([2 * NST, Lc], F32, tag="bcf", bufs=2)
                nc.vector.tensor_copy(bc_f[:], pdbc[0:2 * NST, :])
                bc16 = wp.tile([2 * NST, Lc], BF16, tag="bc16", bufs=2)
                nc.vector.tensor_copy(bc16[:], bc_f[:])

                # ---- F: broadcast B,C rows to 128 partitions (via PE ones)
                Bb, Cb = [], []
                for n in range(NST):
                    pb = ps.tile([128, Lc], F32, tag="ps")
                    nc.tensor.matmul(pb[:], sel[:, n * 128:(n + 1) * 128],
                                     bc16[:], start=True, stop=True)
                    t = wp.tile([128, Lc], BF16, tag=f"bb{n}")
                    nc.vector.tensor_copy(t[:], pb[:])
                    Bb.append(t)
                for n in range(NST):
                    pc = ps.tile([128, Lc], F32, tag="ps")
                    nc.tensor.matmul(pc[:], sel[:, (NST + n) * 128:(NST + n + 1) * 128],
                                     bc16[:], start=True, stop=True)
                    t = wp.tile([128, Lc], BF16, tag=f"cbn{n}")
                    nc.scalar.activation(t[:], pc[:], AF.Identity,
                                         bias=0.0, scale=1.0)
                    Cb.append(t)

                # ---- E+G: per block: dt_proj, softplus, scan over 16 states
                for b in range(NB):
                    pdt = ps.tile([128, Lc], F32, tag="ps")
                    nc.tensor.matmul(pdt[:], wdt_sb[:, b * 128:(b + 1) * 128],
                                     dtlo[:], start=True, stop=True)
                    # softplus(x) = ln(exp(x) + 1); exp+ln share one ACT table
                    spe = wp.tile([128, Lc], F32, tag="spe", bufs=2)
                    i_e = nc.scalar.activation(spe[:], pdt[:], AF.Exp,
                                               bias=bdt_sb[:, b:b + 1], scale=1.0)
                    if silu_chain is not None:
                        add_dep_helper(i_e.ins, silu_chain.ins,
                                       reason="ACT table grouping")
                    dt_f = wp.tile([128, Lc], F32, tag="dt", bufs=3)
                    nc.scalar.activation(dt_f[:], spe[:], AF.Ln,
                                         bias=1.0, scale=1.0)
                    w_f = wp.tile([128, Lc], F32, tag="w", bufs=3)
                    nc.vector.tensor_tensor(w_f[:], dt_f[:], uc_sb[b][:],
                                            op=AL.mult)
                    py = ps.tile([128, Lc], F32, tag="ps")
                    for n in range(NST):
                        dA = wp.tile([128, Lc], F32, tag="dA", bufs=4)
                        i_x = nc.scalar.activation(dA[:], dt_f[:], AF.Exp,
                                                   scale=float(a_vals[n]))
                        if b == NB - 1 and n == NST - 1:
                            prev_last_exp = i_x
                        bn = wp.tile([128, Lc], F32, tag="bn", bufs=3)
                        nc.vector.tensor_tensor(bn[:], w_f[:], Bb[n][:],
                                                op=AL.mult)
                        hn = wp.tile([128, Lc], BF16, tag="hn", bufs=3)
                        j = b * NST + n
                        init = 0.0 if c == 0 else carry[:, j:j + 1]
                        nc.vector.tensor_tensor_scan(
                            hn[:], dA[:], bn[:], init,
                            op0=AL.mult, op1=AL.add)
                        nc.vector.tensor_copy(carry[:, j:j + 1], hn[:, Lc - 1:Lc])
                        gn = wp.tile([128, Lc], BF16, tag="gn", bufs=3)
                        nc.vector.tensor_tensor(gn[:], hn[:], Cb[n][:], op=AL.mult)
                        nc.tensor.matmul(py[:], idn[:], gn[:],
                                         start=(n == 0), stop=(n == NST - 1))

                    # ---- H: gate: sum_t (y + D*uc) * silu(z)
                    yd = wp.tile([128, Lc], F32, tag="yd", bufs=2)
                    nc.vector.scalar_tensor_tensor(
                        yd[:], uc_sb[b][:], dv_sb[:, b:b + 1], py[:],
                        op0=AL.mult, op1=AL.add)
                    junk = wp.tile([128, Lc], F32, tag="junk", bufs=2)
                    nc.vector.scalar_tensor_tensor(
                        junk[:], yd[:], 1.0, sz_sb[b][:],
                        op0=AL.bypass, op1=AL.mult,
                        accum_out=ycol[:, c * NB + b:c * NB + b + 1])

            # ---- FINAL: reduce over chunks, out = (sum_t y)/L @ w_out.T
            ybar = wp.tile([128, NB], F32, tag="ybar")
            yv = ycol[:].rearrange("p (c b) -> p b c", b=NB)
            for b in range(NB):
                nc.vector.tensor_reduce(ybar[:, b:b + 1], yv[:, b:b + 1, :],
                                        axis=mybir.AxisListType.X, op=AL.add)
            # out_proj happens on host (0.002% of FLOPs): ship ybar [128, NB]
            nc.sync.dma_start(out_d.ap().rearrange("(b p) -> p b", p=128),
                              ybar[:])

    nc.compile()
    return nc


_CACHE = {}


def _get_nc(a_vals):
    key = tuple(np.asarray(a_vals, np.float32).tolist())
    if key not in _CACHE:
        _CACHE[key] = build_nc(key)
    return _CACHE[key]


def _sel_matrix():
    s = np.zeros((2 * NST, 2 * NST * 128), np.float32)
    for n in range(2 * NST):
        s[n, n * 128:(n + 1) * 128] = 1.0
    return s.astype(BF)


def prep_inputs(x, w_ip, b_ip, w_in, conv_w, conv_b, w_xp, w_dt, b_dt,
                A_log, D, w_out):
    """Host-side prep: layouts + dtypes. Returns (a_vals, shared_map, xTs)."""
    A = -np.exp(np.asarray(A_log, np.float64))
    assert np.allclose(A, A[0:1, :], rtol=1e-6, atol=1e-9), \
        "kernel assumes A rows identical (S4D init)"
    a_vals = A[0].astype(np.float32)

    shared = {
        "wip": np.ascontiguousarray(np.asarray(w_ip).T).astype(BF),
        "bip": np.ascontiguousarray(np.asarray(b_ip).reshape(MH, 128).T
                                    ).astype(np.float32),
        "win": np.ascontiguousarray(np.asarray(w_in).T).astype(BF),
        "cw": np.ascontiguousarray(
            np.asarray(conv_w).reshape(NB, 128, DCONV).transpose(1, 0, 2)
            .reshape(128, NB * DCONV)).astype(np.float32),
        "cb": np.ascontiguousarray(np.asarray(conv_b).reshape(NB, 128).T
                                   ).astype(np.float32),
        "wxp": np.ascontiguousarray(
            np.concatenate([np.asarray(w_xp)[DTR:DTR + 2 * NST],
                            np.asarray(w_xp)[0:DTR]], axis=0).T).astype(BF),
        "wdt": np.ascontiguousarray(np.asarray(w_dt).T).astype(BF),
        "bdt": np.ascontiguousarray(np.asarray(b_dt).reshape(NB, 128).T
                                    ).astype(np.float32),
        "dv": np.ascontiguousarray(np.asarray(D).reshape(NB, 128).T
                                   ).astype(np.float32),
        "sel": _sel_matrix(),
    }
    x = np.asarray(x)
    xTs = [np.ascontiguousarray(x[c].T).astype(BF) for c in range(B)]
    return a_vals, shared, xTs


def make_in_maps(**inputs):
    a_vals, shared, xTs = prep_inputs(**inputs)
    return a_vals, [{**shared, "xT": xTs[c]} for c in range(B)]


def kernel(**inputs):
    a_vals, in_maps = make_in_maps(**inputs)
    nc = _get_nc(a_vals)
    res = run_bass_kernel_spmd(nc, in_maps, core_ids=list(range(B)))
    ybars = np.stack([res.results[c]["out"] for c in range(B)])  # (B, DI)
    w_out = np.asarray(inputs["w_out"], np.float32)
    return (ybars / np.float32(L)) @ w_out.T.astype(np.float32)

